# revision 1
# baseline (speedup 1.0000x reference)
"""Expert-parallel grouped GEMM (MoE) kernel for Trainium2.

Problem: out[e] = gelu(tok[e] @ w1[e]) @ w2[e]  per expert e.
  tok: [128, 2048, 128] f32, w1: [128, 128, 512] f32, w2: [128, 512, 128] f32.

Sharding: expert-parallel across 8 NeuronCores, 16 experts per core, no
cross-core communication. Each core runs the same Bass program on its own
expert slice (SPMD), the host concatenates the per-core outputs.

Per-core dataflow:
  - tokens loaded naturally ([t,d] tiles), PE-transposed to [d,t] (contraction
    dim of GEMM1 must sit on partitions; fp32 has no DMA-transpose path)
  - GEMM1 in fp32r (full-rate PE, ~13-bit mantissa): hT = w1.T @ tokT
  - GELU (exact/erf) on ScalarE, PSUM -> SBUF, output rounded to fp32r
  - GEMM2 in fp32r: outT[o, t] = sum_hd w2_tile.T @ hT[hd] (PSUM accumulation)
  - PE-transpose outT back to [t, o] in fp32, DMA out naturally
"""

import numpy as np

NUM_CORES = 8
E_TOTAL = 128
E_PER_CORE = E_TOTAL // NUM_CORES  # 16
T = 2048
D = 128
H = 512
O = 128
P = 128

T_CHUNK = 512  # tokens per GEMM moving-operand chunk
N_CHUNKS = T // T_CHUNK  # 4
BLKS_PER_CHUNK = T_CHUNK // P  # 4
N_BLKS = T // P  # 16

_CACHE = {}


DEFAULT_CFG = dict(
    gelu_pair=True,  # True: one [128, 2*512] psum + one gelu per hd pair
    gelu_quad=False,
    chunked_tok=True,
    chunk_first_only=True,  # chunk-granular token load only for expert 0 (startup)
    chunked_out=True,  # per-chunk output stores (earlier store start, smaller tail)
    pt_bufs=2,
    ph_bufs=2,
    po_bufs=1,
    pot_bufs=1,
    tokt_bufs=4,
    h_bufs=8,
    osb_bufs=3,
    tokn_bufs=6,
    outn_bufs=6,
    w_bufs=2,
)


def _build(loop=1, cfg=None):
    import concourse.bacc as bacc
    import concourse.mybir as mybir
    import concourse.tile as tile
    from concourse.masks import make_identity

    f32 = mybir.dt.float32
    f32r = mybir.dt.float32r
    GELU = mybir.ActivationFunctionType.Gelu
    C = dict(DEFAULT_CFG)
    if cfg:
        C.update(cfg)

    nc = bacc.Bacc(
        "TRN2",
        target_bir_lowering=False,
        debug=False,
        num_devices=NUM_CORES,
    )

    tok = nc.dram_tensor(
        "group_token", [E_PER_CORE, T, D], f32, kind="ExternalInput"
    ).ap()
    w1 = nc.dram_tensor("weights1", [E_PER_CORE, D, H], f32, kind="ExternalInput").ap()
    w2 = nc.dram_tensor("weights2", [E_PER_CORE, H, O], f32, kind="ExternalInput").ap()
    out = nc.dram_tensor("out", [E_PER_CORE, T, O], f32, kind="ExternalOutput").ap()

    H_TILES = H // P  # 4

    with tile.TileContext(nc) as tc:
        with (
            tc.tile_pool(name="const", bufs=1) as const_pool,
            tc.tile_pool(name="weights", bufs=C["w_bufs"]) as w_pool,
            tc.tile_pool(name="tokn", bufs=C["tokn_bufs"]) as tokn_pool,
            tc.tile_pool(name="tokt", bufs=C["tokt_bufs"]) as tokt_pool,
            tc.tile_pool(name="hts", bufs=C["h_bufs"]) as h_pool,
            tc.tile_pool(name="osb", bufs=C["osb_bufs"]) as osb_pool,
            tc.tile_pool(name="outn", bufs=C["outn_bufs"]) as outn_pool,
            tc.tile_pool(name="pt", bufs=C["pt_bufs"], space="PSUM") as pt_pool,
            tc.tile_pool(name="ph", bufs=C["ph_bufs"], space="PSUM") as ph_pool,
            tc.tile_pool(name="po", bufs=C["po_bufs"], space="PSUM") as po_pool,
            tc.tile_pool(name="pot", bufs=C["pot_bufs"], space="PSUM") as pot_pool,
        ):
            ident_f32 = const_pool.tile([P, P], f32)
            make_identity(nc, ident_f32)
            ident = const_pool.tile([P, P], f32r)
            nc.vector.tensor_copy(ident[:], ident_f32[:])

            def body(_iv=None):
                for e in range(E_PER_CORE):
                    # tokens: partition p holds the 16 consecutive tokens
                    # t = p*16 + m (m = 4c + j), so both the token load and the
                    # output store are 8 KiB-contiguous per partition.
                    # For expert 0 the chunk-0 token DMA is emitted before the
                    # weight DMAs: SWDGE descriptor generation is serial on the
                    # GpSimd Q7 and the transposes need tokens first.
                    chunk_this = C.get("chunked_tok") and (
                        e == 0 or not C.get("chunk_first_only")
                    )
                    tokn_chunks = None
                    if chunk_this:
                        tokn_chunks = []
                        for cc in range(N_CHUNKS):
                            tkc = tokn_pool.tile(
                                [P, BLKS_PER_CHUNK, D], f32r, tag="tokc", name=f"tokc{e}_{cc}"
                            )
                            nc.gpsimd.dma_start(
                                tkc[:],
                                tok[e].rearrange(
                                    "(p c j) d -> c p j d", c=N_CHUNKS, p=P
                                )[cc],
                            )
                            tokn_chunks.append(tkc)
                    # w1[e]: [128 d, 512 hd] natural; cast-round to f32r in DMA
                    w1_sb = w_pool.tile([P, H], f32r, tag="w1")
                    nc.gpsimd.dma_start(w1_sb[:], w1[e])
                    # w2[e]: [512 hd, 128 o] -> 4 k-tiles [128, 128] on partitions
                    w2_sb = w_pool.tile([P, H_TILES, O], f32r, tag="w2")
                    nc.gpsimd.dma_start(
                        w2_sb[:], w2[e].rearrange("(k p) o -> p k o", p=P)
                    )
                    if not chunk_this:
                        tokn_full = tokn_pool.tile([P, N_BLKS, D], f32r, tag="tokf")
                        nc.gpsimd.dma_start(
                            tokn_full[:], tok[e].rearrange("(p m) d -> p m d", p=P)
                        )
                    # output staging, same layout as tokn
                    if not C.get("chunked_out"):
                        outn = outn_pool.tile([P, N_BLKS, O], f32)

                    for c in range(N_CHUNKS):
                        if chunk_this:
                            blk = lambda j, _t=tokn_chunks[c]: _t[:, j]
                        else:
                            blk = lambda j: tokn_full[:, c * BLKS_PER_CHUNK + j]
                        # transpose 4 token blocks into one psum tile -> tokT [d, 512 t]
                        pt = pt_pool.tile([P, T_CHUNK], f32r)
                        for j in range(BLKS_PER_CHUNK):
                            nc.tensor.transpose(
                                pt[:, j * P : (j + 1) * P], blk(j), ident[:]
                            )
                        tokt = tokt_pool.tile([P, T_CHUNK], f32r)
                        nc.vector.tensor_copy(tokt[:], pt[:])

                        # GEMM1 + GELU: hT[hd_tile] = gelu(w1_slice.T @ tokT)
                        ht_slices = []
                        if C.get("gelu_quad"):
                            ph = ph_pool.tile([P, H_TILES, T_CHUNK], f32)
                            for hd in range(H_TILES):
                                nc.tensor.matmul(
                                    ph[:, hd],
                                    w1_sb[:, hd * P : (hd + 1) * P],
                                    tokt[:],
                                    start=True,
                                    stop=True,
                                )
                            ht = h_pool.tile([P, H_TILES, T_CHUNK], f32r, tag="ht")
                            nc.scalar.activation(ht[:], ph[:], GELU)
                            ht_slices = [ht[:, hd] for hd in range(H_TILES)]
                        elif C["gelu_pair"]:
                            for hp in range(H_TILES // 2):
                                ph = ph_pool.tile([P, 2, T_CHUNK], f32)
                                for k in range(2):
                                    hd = hp * 2 + k
                                    nc.tensor.matmul(
                                        ph[:, k],
                                        w1_sb[:, hd * P : (hd + 1) * P],
                                        tokt[:],
                                        start=True,
                                        stop=True,
                                    )
                                ht = h_pool.tile([P, 2, T_CHUNK], f32r, tag="ht")
                                nc.scalar.activation(ht[:], ph[:], GELU)
                                ht_slices.extend([ht[:, 0], ht[:, 1]])
                        else:
                            for hd in range(H_TILES):
                                ph = ph_pool.tile([P, T_CHUNK], f32)
                                nc.tensor.matmul(
                                    ph[:],
                                    w1_sb[:, hd * P : (hd + 1) * P],
                                    tokt[:],
                                    start=True,
                                    stop=True,
                                )
                                ht = h_pool.tile([P, T_CHUNK], f32r, tag="ht")
                                nc.scalar.activation(ht[:], ph[:], GELU)
                                ht_slices.append(ht[:])

                        # GEMM2: outT[o, t] = sum_hd w2_tile.T @ hT[hd]
                        po = po_pool.tile([P, T_CHUNK], f32)
                        for hd in range(H_TILES):
                            nc.tensor.matmul(
                                po[:],
                                w2_sb[:, hd],
                                ht_slices[hd],
                                start=(hd == 0),
                                stop=(hd == H_TILES - 1),
                            )
                        osb = osb_pool.tile([P, T_CHUNK], f32r)
                        if C.get("osb_alt") and c % 2 == 1:
                            nc.scalar.copy(osb[:], po[:])
                        else:
                            nc.vector.tensor_copy(osb[:], po[:])

                        # transpose back: [o, t] -> [t, o] per 128-token block
                        pot = pot_pool.tile([P, T_CHUNK], f32r)
                        for j in range(BLKS_PER_CHUNK):
                            nc.tensor.transpose(
                                pot[:, j * P : (j + 1) * P],
                                osb[:, j * P : (j + 1) * P],
                                ident[:],
                            )
                        if C.get("chunked_out"):
                            oc = outn_pool.tile([P, BLKS_PER_CHUNK, O], f32, tag="oc")
                            nc.vector.tensor_copy(
                                oc[:],
                                pot[:].rearrange("p (j o) -> p j o", j=BLKS_PER_CHUNK),
                            )
                            nc.sync.dma_start(
                                out[e].rearrange(
                                    "(p c j) o -> c p j o", c=N_CHUNKS, p=P
                                )[c],
                                oc[:],
                            )
                        else:
                            nc.vector.tensor_copy(
                                outn[:, c * BLKS_PER_CHUNK : (c + 1) * BLKS_PER_CHUNK],
                                pot[:].rearrange("p (j o) -> p j o", j=BLKS_PER_CHUNK),
                            )

                    if not C.get("chunked_out"):
                        nc.sync.dma_start(
                            out[e].rearrange("(p m) o -> p m o", p=P), outn[:]
                        )

            def body_swpipe(_iv=None):
                """Software-pipelined emission: next chunk's token transposes are
                interleaved between this chunk's matmuls so transpose weight
                loads hide under matmul streaming (LDW is per-matmul for 4-byte
                stationaries and the cost model does not show this)."""
                NG = E_PER_CORE * N_CHUNKS  # 64 global chunks
                state = {}  # e -> (w1_sb, w2_sb, tok_tiles)

                def setup(e):
                    w1_sb = w_pool.tile([P, H], f32r, tag="w1", name=f"w1s{e}")
                    nc.gpsimd.dma_start(w1_sb[:], w1[e])
                    w2_sb = w_pool.tile([P, H_TILES, O], f32r, tag="w2", name=f"w2s{e}")
                    nc.gpsimd.dma_start(
                        w2_sb[:], w2[e].rearrange("(k p) o -> p k o", p=P)
                    )
                    if e == 0:
                        toks = []
                        for c in range(N_CHUNKS):
                            tk = tokn_pool.tile([P, BLKS_PER_CHUNK, D], f32r, tag="tokc", name=f"tokc{c}")
                            nc.gpsimd.dma_start(
                                tk[:],
                                tok[e].rearrange(
                                    "(p c j) d -> c p j d", c=N_CHUNKS, p=P
                                )[c],
                            )
                            toks.append(tk)
                    else:
                        tf = tokn_pool.tile([P, N_BLKS, D], f32r, tag="tokf", name=f"tokf{e}")
                        nc.gpsimd.dma_start(
                            tf[:], tok[e].rearrange("(p m) d -> p m d", p=P)
                        )
                        toks = tf
                    state[e] = (w1_sb, w2_sb, toks)

                def blk(g, j):
                    e, c = divmod(g, N_CHUNKS)
                    toks = state[e][2]
                    if isinstance(toks, list):
                        return toks[c][:, j]
                    return toks[:, c * BLKS_PER_CHUNK + j]

                pts = {}
                tokts = {}
                hts = {}
                pos = {}
                osbs = {}
                pots = {}

                def tin(g, j):
                    if j == 0:
                        pts[g] = pt_pool.tile([P, T_CHUNK], f32r, tag="pt", name=f"pt{g}")
                    nc.tensor.transpose(
                        pts[g][:, j * P : (j + 1) * P], blk(g, j), ident[:]
                    )

                def tout(g, j):
                    if j == 0:
                        pots[g] = pot_pool.tile([P, T_CHUNK], f32r, tag="pot", name=f"pot{g}")
                    nc.tensor.transpose(
                        pots[g][:, j * P : (j + 1) * P],
                        osbs[g][:, j * P : (j + 1) * P],
                        ident[:],
                    )

                def drain_out(g):
                    e, c = divmod(g, N_CHUNKS)
                    oc = outn_pool.tile([P, BLKS_PER_CHUNK, O], f32, tag="oc", name=f"oc{g}")
                    nc.vector.tensor_copy(
                        oc[:],
                        pots.pop(g)[:].rearrange("p (j o) -> p j o", j=BLKS_PER_CHUNK),
                    )
                    nc.sync.dma_start(
                        out[e].rearrange("(p c j) o -> c p j o", c=N_CHUNKS, p=P)[c],
                        oc[:],
                    )

                setup(0)
                for j in range(BLKS_PER_CHUNK):
                    tin(0, j)

                for g in range(NG):
                    e, c = divmod(g, N_CHUNKS)
                    if c == 2 and e + 1 < E_PER_CORE:
                        setup(e + 1)
                    w1_sb, w2_sb, _ = state[e]

                    tokts[g] = tokt_pool.tile([P, T_CHUNK], f32r, tag="tokt", name=f"tokt{g}")
                    nc.vector.tensor_copy(tokts[g][:], pts.pop(g)[:])

                    # MM1s interleaved with previous chunk's out-transposes
                    ht_slices = []
                    ph = None
                    for hd in range(H_TILES):
                        if hd % 2 == 0:
                            ph = ph_pool.tile([P, 2, T_CHUNK], f32, tag="ph", name=f"ph{g}_{hd}")
                        nc.tensor.matmul(
                            ph[:, hd % 2],
                            w1_sb[:, hd * P : (hd + 1) * P],
                            tokts[g][:],
                            start=True,
                            stop=True,
                        )
                        if g >= 1:
                            tout(g - 1, hd)
                        if hd % 2 == 1:
                            ht = h_pool.tile([P, 2, T_CHUNK], f32r, tag="ht", name=f"ht{g}_{hd}")
                            nc.scalar.activation(ht[:], ph[:], GELU)
                            ht_slices.extend([ht[:, 0], ht[:, 1]])
                    hts[g] = ht_slices
                    if g >= 1:
                        drain_out(g - 1)

                    # MM2s interleaved with next chunk's in-transposes
                    pos[g] = po_pool.tile([P, T_CHUNK], f32, tag="po", name=f"po{g}")
                    for hd in range(H_TILES):
                        nc.tensor.matmul(
                            pos[g][:],
                            w2_sb[:, hd],
                            hts[g][hd],
                            start=(hd == 0),
                            stop=(hd == H_TILES - 1),
                        )
                        if g + 1 < NG:
                            tin(g + 1, hd)
                    osbs[g] = osb_pool.tile([P, T_CHUNK], f32r, tag="osb", name=f"osb{g}")
                    nc.vector.tensor_copy(osbs[g][:], pos.pop(g)[:])
                    tokts.pop(g)

                for j in range(BLKS_PER_CHUNK):
                    tout(NG - 1, j)
                drain_out(NG - 1)

            chosen = body_swpipe if C.get("sw_pipe") else body
            if loop == 1:
                chosen()
            else:
                with tc.For_i(0, loop, 1) as _i:
                    chosen(_i)

    nc.compile()
    return nc


def _get_nc(loop=1, cfg=None):
    key = ("nc", loop, tuple(sorted((cfg or {}).items())))
    if key not in _CACHE:
        _CACHE[key] = _build(loop, cfg)
    return _CACHE[key]


def kernel(group_token, weights1, weights2):
    from concourse.bass_utils import run_bass_kernel_spmd

    group_token = np.ascontiguousarray(np.asarray(group_token, dtype=np.float32))
    weights1 = np.ascontiguousarray(np.asarray(weights1, dtype=np.float32))
    weights2 = np.ascontiguousarray(np.asarray(weights2, dtype=np.float32))

    nc = _get_nc()
    in_maps = []
    for c in range(NUM_CORES):
        sl = slice(c * E_PER_CORE, (c + 1) * E_PER_CORE)
        in_maps.append(
            {
                "group_token": np.ascontiguousarray(group_token[sl]),
                "weights1": np.ascontiguousarray(weights1[sl]),
                "weights2": np.ascontiguousarray(weights2[sl]),
            }
        )

    res = run_bass_kernel_spmd(nc, in_maps, core_ids=list(range(NUM_CORES)))
    _CACHE["last_results"] = res
    return np.concatenate([r["out"] for r in res.results], axis=0)



# revision 4
# speedup vs baseline: 1.2934x; 1.2934x over previous
"""Expert-parallel grouped GEMM (MoE) kernel for Trainium2.

Problem: out[e] = gelu(tok[e] @ w1[e]) @ w2[e]  per expert e.
  tok: [128, 2048, 128] f32, w1: [128, 128, 512] f32, w2: [128, 512, 128] f32.

Sharding: expert-parallel across 8 NeuronCores, 16 experts per core, no
cross-core communication. Each core runs the same Bass program on its own
expert slice (SPMD), the host concatenates the per-core outputs.

v2 dataflow (per core, per 512-token chunk):
  - tokens loaded via casting SWDGE DMA straight to bf16, natural [t, d]
    blocks (partition = t within a 128-token block)
  - PE-transpose token blocks to tokT [d, t] (bf16, 1 cyc/row), DVE copies
    PSUM -> SBUF (2x mode)
  - MM1 (bf16): hT[hd, t] = w1b.T @ tokT, into pair PSUM tiles [128, 2, 512]
  - GELU pair ops on ScalarE: PSUM f32 -> SBUF bf16 ht tiles
  - MM2 (bf16): po[t, o] += ht[hd-slice, t-block].T @ w2b[hd-slice]
    -- ht slices act as the (transposed-consumed) stationary, so the output
    lands in natural [t, o] layout: no output transposes at all
  - Pool drains po PSUM -> SBUF f32, SP HWDGE stores natural [t, o]
  - weights: f32 via SP HWDGE, DVE-cast to bf16 per expert
"""

import numpy as np

NUM_CORES = 8
E_TOTAL = 128
E_PER_CORE = E_TOTAL // NUM_CORES  # 16
T = 2048
D = 128
H = 512
O = 128
P = 128

T_CHUNK = 512
N_CHUNKS = T // T_CHUNK  # 4
BLKS = T_CHUNK // P  # 4 token blocks per chunk
H_TILES = H // P  # 4

_CACHE = {}


DEFAULT_CFG = dict(
    tokb_bufs=3,
    tokc_bufs=4,
    tokt_bufs=3,
    ht_bufs=4,
    oc_bufs=4,
    w_bufs=2,
    pt_bufs=2,
    ph_bufs=2,
    po_bufs=2,
)


def _build(loop=1, cfg=None):
    import concourse.bacc as bacc
    import concourse.mybir as mybir
    import concourse.tile as tile
    from concourse.masks import make_identity

    f32 = mybir.dt.float32
    bf16 = mybir.dt.bfloat16
    GELU = mybir.ActivationFunctionType.Gelu
    C = dict(DEFAULT_CFG)
    if cfg:
        C.update(cfg)

    nc = bacc.Bacc(
        "TRN2",
        target_bir_lowering=False,
        debug=False,
        num_devices=NUM_CORES,
    )

    tok = nc.dram_tensor(
        "group_token", [E_PER_CORE, T, D], f32, kind="ExternalInput"
    ).ap()
    w1 = nc.dram_tensor("weights1", [E_PER_CORE, D, H], f32, kind="ExternalInput").ap()
    w2 = nc.dram_tensor("weights2", [E_PER_CORE, H, O], f32, kind="ExternalInput").ap()
    out = nc.dram_tensor("out", [E_PER_CORE, T, O], f32, kind="ExternalOutput").ap()

    with tile.TileContext(nc) as tc:
        with (
            tc.tile_pool(name="const", bufs=1) as const_pool,
            tc.tile_pool(name="wf", bufs=C["w_bufs"]) as wf_pool,
            tc.tile_pool(name="wb", bufs=C["w_bufs"]) as wb_pool,
            tc.tile_pool(name="tokb", bufs=C["tokb_bufs"]) as tokb_pool,
            tc.tile_pool(name="tokc", bufs=C["tokc_bufs"]) as tokc_pool,
            tc.tile_pool(name="tokt", bufs=C["tokt_bufs"]) as tokt_pool,
            tc.tile_pool(name="ht", bufs=C["ht_bufs"]) as ht_pool,
            tc.tile_pool(name="oc", bufs=C["oc_bufs"]) as oc_pool,
            tc.tile_pool(name="pt", bufs=C["pt_bufs"], space="PSUM") as pt_pool,
            tc.tile_pool(name="ph", bufs=C["ph_bufs"], space="PSUM") as ph_pool,
            tc.tile_pool(name="po", bufs=C["po_bufs"], space="PSUM") as po_pool,
        ):
            ident_f32 = const_pool.tile([P, P], f32)
            make_identity(nc, ident_f32)
            ident = const_pool.tile([P, P], bf16)
            nc.vector.tensor_copy(ident[:], ident_f32[:])

            NG = E_PER_CORE * N_CHUNKS  # 64 global chunks

            def body(_iv=None):
                state = {}  # e -> (w1b, w2b, toks)

                def setup(e):
                    # weights f32 via SP HWDGE, then DVE cast to bf16
                    w1f = wf_pool.tile([P, H], f32, tag="w1f", name=f"w1f{e}")
                    nc.sync.dma_start(w1f[:], w1[e])
                    w2f = wf_pool.tile([P, H_TILES, O], f32, tag="w2f", name=f"w2f{e}")
                    nc.sync.dma_start(w2f[:], w2[e].rearrange("(k p) o -> p k o", p=P))
                    w1b = wb_pool.tile([P, H], bf16, tag="w1b", name=f"w1b{e}")
                    nc.vector.tensor_copy(w1b[:], w1f[:])
                    w2b = wb_pool.tile([P, H_TILES, O], bf16, tag="w2b", name=f"w2b{e}")
                    nc.vector.tensor_copy(w2b[:], w2f[:])
                    # tokens: casting DMA (gpsimd SWDGE) f32 -> bf16, natural
                    # blocks: partition = t within block, block (c, j)
                    if e == 0:
                        toks = []
                        for c in range(N_CHUNKS):
                            tkc = tokc_pool.tile(
                                [P, BLKS, D], bf16, tag="tokc", name=f"tokc{c}"
                            )
                            nc.gpsimd.dma_start(
                                tkc[:],
                                tok[e].rearrange(
                                    "(c j p) d -> c p j d", c=N_CHUNKS, j=BLKS, p=P
                                )[c],
                            )
                            toks.append(tkc)
                    else:
                        tf = tokb_pool.tile(
                            [P, N_CHUNKS * BLKS, D], bf16, tag="tokb", name=f"tokb{e}"
                        )
                        nc.gpsimd.dma_start(
                            tf[:],
                            tok[e].rearrange("(m p) d -> p m d", p=P),
                        )
                        toks = tf
                    state[e] = (w1b, w2b, toks)

                def blk(g, j):
                    e, c = divmod(g, N_CHUNKS)
                    toks = state[e][2]
                    if isinstance(toks, list):
                        return toks[c][:, j]
                    return toks[:, c * BLKS + j]

                pts = {}
                hts = {}
                pos = {}

                def tin(g, j):
                    # PE transpose token block j of chunk g into pt[g]
                    if j == 0:
                        pts[g] = pt_pool.tile(
                            [P, T_CHUNK], bf16, tag="pt", name=f"pt{g}"
                        )
                    nc.tensor.transpose(
                        pts[g][:, j * P : (j + 1) * P], blk(g, j), ident[:]
                    )

                def mm2(g, b):
                    # po[:, b, :] += ht[hd].T @ w2b[hd] over 4 hd tiles
                    e = g // N_CHUNKS
                    w2b = state[e][1]
                    hta, htb = hts[g]
                    for k in range(H_TILES):
                        src = hta if k < 2 else htb
                        nc.tensor.matmul(
                            pos[g][:, b],
                            src[:, k % 2, b * P : (b + 1) * P],
                            w2b[:, k],
                            start=(k == 0),
                            stop=(k == H_TILES - 1),
                        )

                def drain(g):
                    e, c = divmod(g, N_CHUNKS)
                    oc = oc_pool.tile([P, BLKS, O], f32, tag="oc", name=f"oc{g}")
                    nc.vector.tensor_copy(oc[:], pos.pop(g)[:])
                    nc.sync.dma_start(
                        out[e].rearrange("(c b p) o -> c p b o", c=N_CHUNKS, p=P)[c],
                        oc[:],
                    )
                    hts.pop(g)

                setup(0)
                for j in range(BLKS):
                    tin(0, j)

                for g in range(NG):
                    e, c = divmod(g, N_CHUNKS)
                    if c == 2 and e + 1 < E_PER_CORE:
                        setup(e + 1)
                    w1b = state[e][0]

                    tokt = tokt_pool.tile(
                        [P, T_CHUNK], bf16, tag="tokt", name=f"tokt{g}"
                    )
                    nc.vector.tensor_copy(tokt[:], pts.pop(g)[:])

                    # MM1 in two pair-PSUM tiles, GELU after each pair
                    pair_tiles = []
                    for hp in range(H_TILES // 2):
                        ph = ph_pool.tile(
                            [P, 2, T_CHUNK], f32, tag="ph", name=f"ph{g}_{hp}"
                        )
                        for k in range(2):
                            hd = hp * 2 + k
                            nc.tensor.matmul(
                                ph[:, k],
                                w1b[:, hd * P : (hd + 1) * P],
                                tokt[:],
                                start=True,
                                stop=True,
                            )
                        ht = ht_pool.tile(
                            [P, 2, T_CHUNK], bf16, tag="ht", name=f"ht{g}_{hp}"
                        )
                        nc.scalar.activation(ht[:], ph[:], GELU)
                        pair_tiles.append(ht)
                    hts[g] = pair_tiles

                    # next chunk's token transposes (hide under MM stream)
                    if g + 1 < NG:
                        for j in range(BLKS):
                            tin(g + 1, j)

                    # MM2 for previous chunk (its GELU finished last round)
                    if g >= 1:
                        pos[g - 1] = po_pool.tile(
                            [P, BLKS, O], f32, tag="po", name=f"po{g - 1}"
                        )
                        for b in range(BLKS):
                            mm2(g - 1, b)
                        drain(g - 1)

                # tail: last chunk
                g = NG - 1
                pos[g] = po_pool.tile([P, BLKS, O], f32, tag="po", name=f"po{g}")
                for b in range(BLKS):
                    mm2(g, b)
                drain(g)

            if loop == 1:
                body()
            else:
                with tc.For_i(0, loop, 1) as _i:
                    body(_i)

    nc.compile()
    return nc


def _get_nc(loop=1, cfg=None):
    key = ("nc", loop, tuple(sorted((cfg or {}).items())))
    if key not in _CACHE:
        _CACHE[key] = _build(loop, cfg)
    return _CACHE[key]


def kernel(group_token, weights1, weights2):
    from concourse.bass_utils import run_bass_kernel_spmd

    group_token = np.ascontiguousarray(np.asarray(group_token, dtype=np.float32))
    weights1 = np.ascontiguousarray(np.asarray(weights1, dtype=np.float32))
    weights2 = np.ascontiguousarray(np.asarray(weights2, dtype=np.float32))

    nc = _get_nc()
    in_maps = []
    for c in range(NUM_CORES):
        sl = slice(c * E_PER_CORE, (c + 1) * E_PER_CORE)
        in_maps.append(
            {
                "group_token": np.ascontiguousarray(group_token[sl]),
                "weights1": np.ascontiguousarray(weights1[sl]),
                "weights2": np.ascontiguousarray(weights2[sl]),
            }
        )

    res = run_bass_kernel_spmd(nc, in_maps, core_ids=list(range(NUM_CORES)))
    _CACHE["last_results"] = res
    return np.concatenate([r["out"] for r in res.results], axis=0)
